# revision 16
# baseline (speedup 1.0000x reference)
"""CosineCrossAttention Trainium2 kernel.

Math (per (b,t)):
    q = query @ Wq                      (N, D), heads head-major: d = h*48+dh
    k = kv @ Wk   (1, D);  v = kv @ Wv  (1, D)
    attn[n,h] = (q_h . k_h) / (|q_h||k_h|)
    out[n, dh*8+h] = attn[n,h] * v[h,dh];  out = out @ Wp + bp

Restructured:
    k_scaled = k / |k_h|  per head
    Kmat[d,h]  = k_scaled[d] * (d//48 == h)          (D, H)
    Wqk        = Wq @ Kmat                           (D, H)
    attn_raw   = query @ Wqk        = q.k/|k|        (N, H)
    ss[n,h]    = sum_{d in head h} q[n,d]^2          via mask48 matmul on q^2
    attn       = attn_raw / sqrt(ss)
    v_perm     = kv @ Wv_perm   (Wv columns permuted so v_perm[d] = v[(d%8)*48+d//8])
    Wp_eff[h,:]= sum_d v_perm[d]*(d%8==h)*Wp[d,:]    (H, D)
    out        = attn @ Wp_eff + bp

Everything on-device runs in the transposed domain (D on partitions, N on free):
host passes query^T per (b,t) so matmul contraction dims land on partitions with
contiguous DMA. Sharding: data-parallel over B across the 8 cores.

COMPUTE_DTYPE selects the TensorEngine dtype: "bf16" (fast, ~1e-2 err),
"f32r" (full-rate fp32 streaming mode), or "f32" (exact, 4x slower PE).
PSUM accumulation is always fp32; norms/recip and the output are fp32.
"""

import sys

sys.path.insert(0, "/opt/trn_rl_repo")

from contextlib import ExitStack

import ml_dtypes
import numpy as np

import concourse.bass as bass
import concourse.tile as tile
from concourse import bacc, mybir
from concourse.masks import make_identity

F32 = mybir.dt.float32

B, T, N, D, H, Dh = 8, 8, 2048, 384, 8, 48
P = 128
CH = D // P  # 3 chunks of the D dims
NG = 512  # n-group (one PSUM bank of f32)

COMPUTE_DTYPE = "bf16"  # "bf16" | "f32r" | "f32"

_CDT = {
    "bf16": mybir.dt.bfloat16,
    "f32r": mybir.dt.float32r,
    "f32": mybir.dt.float32,
}
_NPDT = {
    "bf16": ml_dtypes.bfloat16,
    "f32r": np.float32,
    "f32": np.float32,
}


def build_nc(t_dim=T, n_dim=N, ng=NG, cdtype=None):
    cdtype = cdtype or COMPUTE_DTYPE
    CD = _CDT[cdtype]
    ngrp = n_dim // ng
    nc = bacc.Bacc("TRN2", target_bir_lowering=False, debug=False)

    qT = nc.dram_tensor("qT", [t_dim, D, n_dim], CD, kind="ExternalInput").ap()
    kvT = nc.dram_tensor("kvT", [D, t_dim], CD, kind="ExternalInput").ap()
    wq_d = nc.dram_tensor("Wq", [D, D], CD, kind="ExternalInput").ap()
    wqT_d = nc.dram_tensor("WqT", [D, D], CD, kind="ExternalInput").ap()
    wk_d = nc.dram_tensor("Wk", [D, D], CD, kind="ExternalInput").ap()
    wv_d = nc.dram_tensor("Wvp", [D, D], CD, kind="ExternalInput").ap()
    wp_d = nc.dram_tensor("Wp", [D, D], CD, kind="ExternalInput").ap()
    bp_d = nc.dram_tensor("bp", [D], F32, kind="ExternalInput").ap()
    m48_d = nc.dram_tensor("m48", [D, H], CD, kind="ExternalInput").ap()
    mv_d = nc.dram_tensor("mv", [D, H], CD, kind="ExternalInput").ap()
    outT = nc.dram_tensor("outT", [t_dim, D, n_dim], F32, kind="ExternalOutput").ap()

    with tile.TileContext(nc) as tc, ExitStack() as ctx:
        consts = ctx.enter_context(tc.tile_pool(name="consts", bufs=1))
        qpool = ctx.enter_context(tc.tile_pool(name="qpool", bufs=2))
        work = ctx.enter_context(tc.tile_pool(name="work", bufs=2))
        small = ctx.enter_context(tc.tile_pool(name="small", bufs=3))
        psum1 = ctx.enter_context(tc.tile_pool(name="psum1", bufs=1, space="PSUM"))
        psum2 = ctx.enter_context(tc.tile_pool(name="psum2", bufs=2, space="PSUM"))
        psum3 = ctx.enter_context(tc.tile_pool(name="psum3", bufs=3, space="PSUM"))
        dram = ctx.enter_context(tc.tile_pool(name="dram", bufs=1, space="DRAM"))

        # ---------- first query slice: queue its DMA before everything ----------
        qt0 = qpool.tile([P, CH, n_dim], CD, tag="qt")
        for c in range(CH):
            nc.sync.dma_start(qt0[:, c, :], qT[0, c * P : (c + 1) * P, :])

        # ---------- constants (gpsimd queue, off the query path) ----------
        def load_w(dram, tag):
            sb = consts.tile([P, CH, D], CD, tag=tag)
            nc.sync.dma_start(sb, dram.rearrange("(c p) f -> p c f", p=P))
            return sb

        wq_sb = load_w(wq_d, "wq")
        wk_sb = load_w(wk_d, "wk")
        wv_sb = load_w(wv_d, "wv")
        wp_sb = load_w(wp_d, "wp")

        m48_sb = consts.tile([P, CH, H], CD, tag="m48")
        nc.sync.dma_start(m48_sb, m48_d.rearrange("(c p) h -> p c h", p=P))
        mv_sb = consts.tile([P, CH, H], CD, tag="mv")
        nc.sync.dma_start(mv_sb, mv_d.rearrange("(c p) h -> p c h", p=P))
        kvt_sb = consts.tile([P, CH, t_dim], CD, tag="kvt")
        nc.sync.dma_start(kvt_sb, kvT.rearrange("(c p) t -> p c t", p=P))
        bp_sb = consts.tile([P, CH], F32, tag="bp")
        nc.sync.dma_start(bp_sb, bp_d.rearrange("(c p) -> p c", p=P))

        # Wq^T comes pre-transposed from the host
        wqT_sb = load_w(wqT_d, "wqT")

        # ---------- k/v projections for all t ----------
        ps_k = psum2.tile([t_dim, D], F32, tag="par")
        ps_v = psum2.tile([t_dim, D], F32, tag="par")
        for c in range(CH):
            nc.tensor.matmul(
                ps_k, kvt_sb[:, c, :], wk_sb[:, c, :],
                start=(c == 0), stop=(c == CH - 1),
            )
        for c in range(CH):
            nc.tensor.matmul(
                ps_v, kvt_sb[:, c, :], wv_sb[:, c, :],
                start=(c == 0), stop=(c == CH - 1),
            )

        # copies to SBUF (k left unscaled; 1/|k_h| is folded into the tail)
        k_sb = work.tile([t_dim, D], F32, tag="k_sb")
        nc.scalar.copy(k_sb, ps_k)
        vsb = work.tile([t_dim, D], F32, tag="vsb")
        nc.scalar.copy(vsb, ps_v)

        # transpose k, v -> (D-part, t); cast to compute dtype on copyback
        kT = consts.tile([P, CH, t_dim], CD, tag="kT")
        vT = consts.tile([P, CH, t_dim], CD, tag="vT")
        idt = consts.tile([t_dim, t_dim], F32, tag="idt")
        make_identity(nc, idt)
        for c in range(CH):
            pt = psum1.tile([P, t_dim], F32, tag="pq")
            nc.tensor.transpose(pt, k_sb[:, c * P : (c + 1) * P], idt)
            nc.vector.tensor_copy(kT[:, c, :], pt)
            pt2 = psum3.tile([P, t_dim], F32, tag="po")
            nc.tensor.transpose(pt2, vsb[:, c * P : (c + 1) * P], idt)
            nc.vector.tensor_copy(vT[:, c, :], pt2)

        # per-head k norms, transposed domain (off the wqk critical path):
        # rnkT[h, t] = 1/|k_h|(t)
        ksqT = work.tile([P, CH, t_dim], CD, tag="ksqT")
        nc.scalar.square(ksqT, kT)
        psk2 = psum3.tile([H, t_dim], F32, tag="po")
        for c in range(CH):
            nc.tensor.matmul(
                psk2, m48_sb[:, c, :], ksqT[:, c, :],
                start=(c == 0), stop=(c == CH - 1),
            )
        rnkT = consts.tile([H, t_dim], F32, tag="rnkT")
        nc.scalar.sqrt(rnkT, psk2)
        nc.vector.reciprocal(rnkT, rnkT)

        # Kmat[d, t, h] = kT[d, t] * m48[d, h];  Vsel[d, t, h] = vT[d, t] * mv[d, h]
        kmat = consts.tile([P, CH, t_dim, H], CD, tag="kmat")
        nc.vector.tensor_tensor(
            kmat,
            kT[:, :, :, None].to_broadcast((P, CH, t_dim, H)),
            m48_sb[:, :, None, :].to_broadcast((P, CH, t_dim, H)),
            op=mybir.AluOpType.mult,
        )
        vsel = consts.tile([P, CH, t_dim, H], CD, tag="vsel")
        nc.vector.tensor_tensor(
            vsel,
            vT[:, :, :, None].to_broadcast((P, CH, t_dim, H)),
            mv_sb[:, :, None, :].to_broadcast((P, CH, t_dim, H)),
            op=mybir.AluOpType.mult,
        )

        # Wqk[d_in, t, h] = sum_dmid Wq[d_in, dmid] Kmat[dmid, t, h]
        wqk = consts.tile([P, CH, t_dim, H], CD, tag="wqk")
        for ci in range(CH):
            pw = psum2.tile([P, t_dim * H], F32, tag="par")
            for cm in range(CH):
                nc.tensor.matmul(
                    pw,
                    wqT_sb[:, cm, ci * P : (ci + 1) * P],
                    kmat[:, cm, :, :],
                    start=(cm == 0), stop=(cm == CH - 1),
                )
            nc.scalar.copy(wqk[:, ci], pw.rearrange("p (t h) -> p t h", h=H))

        # Wp_eff[(t,h), d_out] = sum_d Vsel[d, t, h] * Wp[d, d_out]  (all t at once)
        pe_all = psum3.tile([t_dim * H, D], F32, tag="po")
        for c in range(CH):
            nc.tensor.matmul(
                pe_all, vsel[:, c].rearrange("p t h -> p (t h)"), wp_sb[:, c, :],
                start=(c == 0), stop=(c == CH - 1),
            )
        wpe_stage = work.tile([t_dim * H, D], CD, tag="wpestage")
        nc.scalar.copy(wpe_stage, pe_all)
        wpe_dram = dram.tile([t_dim * H, D], CD)
        nc.sync.dma_start(wpe_dram, wpe_stage)
        wpe = consts.tile([H, t_dim, D], CD, tag="wpe")
        nc.sync.dma_start(wpe, wpe_dram.rearrange("(t h) d -> h t d", h=H))

        # ---------- main loop (software-pipelined: tail lags head by 1 group) ----------
        def emit_tail(par, t, sl):
            nrm = small.tile([H, ng], F32, tag="nrm")
            nc.scalar.sqrt(nrm, par[32 : 32 + H, :])
            rcp = small.tile([H, ng], F32, tag="rcp")
            nc.vector.reciprocal_approx_fast(rcp, nrm)
            att = small.tile([H, ng], CD, tag="att")
            nc.vector.scalar_tensor_tensor(
                att, par[0:H, :], rnkT[:, t : t + 1], rcp,
                op0=mybir.AluOpType.mult, op1=mybir.AluOpType.mult,
            )
            # out^T chunks = Wp_eff^T @ attn + bp
            osb = work.tile([P, CH, ng], F32, tag="osb")
            for co in range(CH):
                po = psum3.tile([P, ng], F32, tag="po")
                nc.tensor.matmul(
                    po, wpe[:, t, co * P : (co + 1) * P], att,
                    start=True, stop=True,
                )
                if co == 0:
                    nc.scalar.activation(
                        osb[:, co, :], po,
                        mybir.ActivationFunctionType.Identity,
                        bias=bp_sb[:, co : co + 1], scale=1.0,
                    )
                else:
                    nc.vector.tensor_tensor(
                        osb[:, co, :], po,
                        bp_sb[:, co : co + 1].to_broadcast((P, ng)),
                        op=mybir.AluOpType.add,
                    )
            nc.sync.dma_start(
                outT[t].rearrange("(c p) n -> p c n", p=P)[:, :, sl], osb
            )

        pending = None
        for t in range(t_dim):
            qt = qt0 if t == 0 else qpool.tile([P, CH, n_dim], CD, tag="qt")
            if t > 0:
                for c in range(CH):
                    nc.sync.dma_start(qt[:, c, :], qT[t, c * P : (c + 1) * P, :])
            for g in range(ngrp):
                sl = slice(g * ng, (g + 1) * ng)
                # projection chunks; per-chunk square + ss matmul interleave.
                # ss -> rows 32:40, attn_raw -> rows 0:8 of one psum bank.
                par = psum2.tile([40, ng], F32, tag="par")
                pq = psum1.tile([P, CH, ng], F32, tag="pq")
                qsq = work.tile([P, CH, ng], CD, tag="qsq")
                for co in range(CH):
                    for c in range(CH):
                        nc.tensor.matmul(
                            pq[:, co, :],
                            wq_sb[:, c, co * P : (co + 1) * P], qt[:, c, sl],
                            start=(c == 0), stop=(c == CH - 1),
                        )
                    nc.scalar.square(qsq[:, co, :], pq[:, co, :])
                    nc.tensor.matmul(
                        par[32 : 32 + H, :], m48_sb[:, co, :], qsq[:, co, :],
                        start=(co == 0), stop=(co == CH - 1),
                        tile_position=(0, 32),
                    )
                for c in range(CH):
                    nc.tensor.matmul(
                        par[0:H, :], wqk[:, c, t, :], qt[:, c, sl],
                        start=(c == 0), stop=(c == CH - 1),
                        tile_position=(0, 0),
                    )
                if pending is not None:
                    emit_tail(*pending)
                pending = (par, t, sl)
        emit_tail(*pending)

    nc.compile()
    return nc


_CACHE = {}


def _get_nc(t_dim=T, n_dim=N):
    key = (t_dim, n_dim, COMPUTE_DTYPE)
    if key not in _CACHE:
        _CACHE[key] = build_nc(t_dim, n_dim)
    return _CACHE[key]


def _host_prep(query, kv, Wq, Wk, Wv, Wp, bp):
    ndt = _NPDT[COMPUTE_DTYPE]
    query = np.asarray(query, dtype=np.float32)
    kv = np.asarray(kv, dtype=np.float32)
    Wq = np.ascontiguousarray(np.asarray(Wq, dtype=np.float32).astype(ndt))
    WqT = np.ascontiguousarray(Wq.T)
    Wk = np.ascontiguousarray(np.asarray(Wk, dtype=np.float32).astype(ndt))
    Wv = np.asarray(Wv, dtype=np.float32)
    Wp = np.ascontiguousarray(np.asarray(Wp, dtype=np.float32).astype(ndt))
    bp = np.ascontiguousarray(np.asarray(bp, dtype=np.float32))

    b_dim, t_dim, n_dim, d = query.shape
    dh = d // H
    # Wv with columns permuted: v_perm[d] = v[(d%H)*dh + d//H]
    perm = (np.arange(d) % H) * dh + np.arange(d) // H
    Wvp = np.ascontiguousarray(Wv[:, perm].astype(ndt))
    dd = np.arange(d)
    hh = np.arange(H)
    m48 = (dd[:, None] // dh == hh[None, :]).astype(ndt)
    mv = (dd[:, None] % H == hh[None, :]).astype(ndt)

    in_maps = []
    for b in range(b_dim):
        in_maps.append(
            {
                "qT": np.ascontiguousarray(query[b].transpose(0, 2, 1).astype(ndt)),
                "kvT": np.ascontiguousarray(kv[b, :, 0, :].T.astype(ndt)),
                "Wq": Wq,
                "WqT": WqT,
                "Wk": Wk,
                "Wvp": Wvp,
                "Wp": Wp,
                "bp": bp,
                "m48": m48,
                "mv": mv,
            }
        )
    return in_maps, (b_dim, t_dim, n_dim, d)


def _gather(results, shape):
    b_dim, t_dim, n_dim, d = shape
    out = np.empty((b_dim, t_dim, n_dim, d), dtype=np.float32)
    for b in range(b_dim):
        out[b] = results[b]["outT"].transpose(0, 2, 1)
    return out


def kernel(query, kv, Wq, Wk, Wv, Wp, bp):
    from concourse.bass_utils import run_bass_kernel_spmd

    in_maps, shape = _host_prep(query, kv, Wq, Wk, Wv, Wp, bp)
    nc = _get_nc(shape[1], shape[2])
    res = run_bass_kernel_spmd(nc, in_maps, core_ids=list(range(len(in_maps))))
    return _gather(res.results, shape)


def _install_ntff_hook():
    """The agent image's antenv lacks axon_hooks; synthesize it so
    run_bass_kernel_spmd(trace=True) can capture NTFF profiles."""
    import types

    if "antenv.axon_hooks" in sys.modules:
        return
    sys.path.insert(0, "/root/.axon_site")
    from trn_agent_boot.trn_boot import _ntff_profile_via_ctypes

    hook = _ntff_profile_via_ctypes("/opt/axon/libaxon_pjrt.so")
    mod = types.ModuleType("antenv.axon_hooks")
    mod.get_axon_ntff_profile_hook = lambda: hook
    mod.set_axon_ntff_profile_hook = lambda h: None
    sys.modules["antenv.axon_hooks"] = mod


def kernel_traced(query, kv, Wq, Wk, Wv, Wp, bp):
    """Like kernel() but captures an NTFF profile; returns (out, results)."""
    from concourse.bass_utils import run_bass_kernel_spmd

    _install_ntff_hook()
    in_maps, shape = _host_prep(query, kv, Wq, Wk, Wv, Wp, bp)
    nc = _get_nc(shape[1], shape[2])
    res = run_bass_kernel_spmd(
        nc, in_maps, core_ids=list(range(len(in_maps))), trace=True
    )
    return _gather(res.results, shape), res


# revision 17
# speedup vs baseline: 1.1754x; 1.1754x over previous
"""CosineCrossAttention Trainium2 kernel.

Math (per (b,t)):
    q = query @ Wq                      (N, D), heads head-major: d = h*48+dh
    k = kv @ Wk   (1, D);  v = kv @ Wv  (1, D)
    attn[n,h] = (q_h . k_h) / (|q_h||k_h|)
    out[n, dh*8+h] = attn[n,h] * v[h,dh];  out = out @ Wp + bp

Restructured:
    k_scaled = k / |k_h|  per head
    Kmat[d,h]  = k_scaled[d] * (d//48 == h)          (D, H)
    Wqk        = Wq @ Kmat                           (D, H)
    attn_raw   = query @ Wqk        = q.k/|k|        (N, H)
    ss[n,h]    = sum_{d in head h} q[n,d]^2          via mask48 matmul on q^2
    attn       = attn_raw / sqrt(ss)
    v_perm     = kv @ Wv_perm   (Wv columns permuted so v_perm[d] = v[(d%8)*48+d//8])
    Wp_eff[h,:]= sum_d v_perm[d]*(d%8==h)*Wp[d,:]    (H, D)
    out        = attn @ Wp_eff + bp

Everything on-device runs in the transposed domain (D on partitions, N on free):
host passes query^T per (b,t) so matmul contraction dims land on partitions with
contiguous DMA. Sharding: data-parallel over B across the 8 cores.

COMPUTE_DTYPE selects the TensorEngine dtype: "bf16" (fast, ~1e-2 err),
"f32r" (full-rate fp32 streaming mode), or "f32" (exact, 4x slower PE).
PSUM accumulation is always fp32; norms/recip and the output are fp32.
"""

import sys

sys.path.insert(0, "/opt/trn_rl_repo")

from contextlib import ExitStack

import ml_dtypes
import numpy as np

import concourse.bass as bass
import concourse.tile as tile
from concourse import bacc, mybir
from concourse.masks import make_identity

F32 = mybir.dt.float32

B, T, N, D, H, Dh = 8, 8, 2048, 384, 8, 48
P = 128
CH = D // P  # 3 chunks of the D dims
NG = 512  # n-group (one PSUM bank of f32)

COMPUTE_DTYPE = "bf16"  # "bf16" | "f32r" | "f32"

_CDT = {
    "bf16": mybir.dt.bfloat16,
    "f32r": mybir.dt.float32r,
    "f32": mybir.dt.float32,
}
_NPDT = {
    "bf16": ml_dtypes.bfloat16,
    "f32r": np.float32,
    "f32": np.float32,
}


def build_nc(t_dim=T, n_dim=N, ng=NG, cdtype=None):
    cdtype = cdtype or COMPUTE_DTYPE
    CD = _CDT[cdtype]
    ngrp = n_dim // ng
    nc = bacc.Bacc("TRN2", target_bir_lowering=False, debug=False)

    qT = nc.dram_tensor("qT", [t_dim, D, n_dim], CD, kind="ExternalInput").ap()
    kvT = nc.dram_tensor("kvT", [D, t_dim], CD, kind="ExternalInput").ap()
    wq_d = nc.dram_tensor("Wq", [D, D], CD, kind="ExternalInput").ap()
    wqT_d = nc.dram_tensor("WqT", [D, D], CD, kind="ExternalInput").ap()
    wk_d = nc.dram_tensor("Wk", [D, D], CD, kind="ExternalInput").ap()
    wv_d = nc.dram_tensor("Wvp", [D, D], CD, kind="ExternalInput").ap()
    wp_d = nc.dram_tensor("Wp", [D, D], CD, kind="ExternalInput").ap()
    bp_d = nc.dram_tensor("bp", [D], F32, kind="ExternalInput").ap()
    m48_d = nc.dram_tensor("m48", [D, H], CD, kind="ExternalInput").ap()
    mv_d = nc.dram_tensor("mv", [D, H], CD, kind="ExternalInput").ap()
    outT = nc.dram_tensor("outT", [t_dim, D, n_dim], F32, kind="ExternalOutput").ap()

    with tile.TileContext(nc) as tc, ExitStack() as ctx:
        consts = ctx.enter_context(tc.tile_pool(name="consts", bufs=1))
        qpool = ctx.enter_context(tc.tile_pool(name="qpool", bufs=2))
        work = ctx.enter_context(tc.tile_pool(name="work", bufs=2))
        small = ctx.enter_context(tc.tile_pool(name="small", bufs=3))
        psum1 = ctx.enter_context(tc.tile_pool(name="psum1", bufs=1, space="PSUM"))
        psum2 = ctx.enter_context(tc.tile_pool(name="psum2", bufs=2, space="PSUM"))
        psum3 = ctx.enter_context(tc.tile_pool(name="psum3", bufs=3, space="PSUM"))
        dram = ctx.enter_context(tc.tile_pool(name="dram", bufs=1, space="DRAM"))

        # ---------- first query slice: queue its DMA before everything ----------
        qt0 = qpool.tile([P, CH, n_dim], CD, tag="qt")
        for c in range(CH):
            nc.sync.dma_start(qt0[:, c, :], qT[0, c * P : (c + 1) * P, :])

        # ---------- constants (gpsimd queue, off the query path) ----------
        def load_w(dram, tag):
            sb = consts.tile([P, CH, D], CD, tag=tag)
            nc.sync.dma_start(sb, dram.rearrange("(c p) f -> p c f", p=P))
            return sb

        wq_sb = load_w(wq_d, "wq")
        wk_sb = load_w(wk_d, "wk")
        wv_sb = load_w(wv_d, "wv")
        wp_sb = load_w(wp_d, "wp")

        m48_sb = consts.tile([P, CH, H], CD, tag="m48")
        nc.sync.dma_start(m48_sb, m48_d.rearrange("(c p) h -> p c h", p=P))
        mv_sb = consts.tile([P, CH, H], CD, tag="mv")
        nc.sync.dma_start(mv_sb, mv_d.rearrange("(c p) h -> p c h", p=P))
        kvt_sb = consts.tile([P, CH, t_dim], CD, tag="kvt")
        nc.sync.dma_start(kvt_sb, kvT.rearrange("(c p) t -> p c t", p=P))
        bp_sb = consts.tile([P, CH], F32, tag="bp")
        nc.sync.dma_start(bp_sb, bp_d.rearrange("(c p) -> p c", p=P))

        # Wq^T comes pre-transposed from the host
        wqT_sb = load_w(wqT_d, "wqT")

        # ---------- k/v projections for all t ----------
        ps_k = psum2.tile([t_dim, D], F32, tag="par")
        ps_v = psum2.tile([t_dim, D], F32, tag="par")
        for c in range(CH):
            nc.tensor.matmul(
                ps_k, kvt_sb[:, c, :], wk_sb[:, c, :],
                start=(c == 0), stop=(c == CH - 1),
            )
        for c in range(CH):
            nc.tensor.matmul(
                ps_v, kvt_sb[:, c, :], wv_sb[:, c, :],
                start=(c == 0), stop=(c == CH - 1),
            )

        # copies to SBUF (k left unscaled; 1/|k_h| is folded into the tail)
        k_sb = work.tile([t_dim, D], F32, tag="k_sb")
        nc.scalar.copy(k_sb, ps_k)
        vsb = work.tile([t_dim, D], F32, tag="vsb")
        nc.scalar.copy(vsb, ps_v)

        # transpose k, v -> (D-part, t); cast to compute dtype on copyback
        kT = consts.tile([P, CH, t_dim], CD, tag="kT")
        vT = consts.tile([P, CH, t_dim], CD, tag="vT")
        idt = consts.tile([t_dim, t_dim], F32, tag="idt")
        make_identity(nc, idt)
        for c in range(CH):
            pt = psum1.tile([P, t_dim], F32, tag="pq")
            nc.tensor.transpose(pt, k_sb[:, c * P : (c + 1) * P], idt)
            nc.vector.tensor_copy(kT[:, c, :], pt)
            pt2 = psum3.tile([P, t_dim], F32, tag="po")
            nc.tensor.transpose(pt2, vsb[:, c * P : (c + 1) * P], idt)
            nc.vector.tensor_copy(vT[:, c, :], pt2)

        # per-head k norms, transposed domain (off the wqk critical path):
        # rnkT[h, t] = 1/|k_h|(t)
        ksqT = work.tile([P, CH, t_dim], CD, tag="ksqT")
        nc.scalar.square(ksqT, kT)
        psk2 = psum3.tile([H, t_dim], F32, tag="po")
        for c in range(CH):
            nc.tensor.matmul(
                psk2, m48_sb[:, c, :], ksqT[:, c, :],
                start=(c == 0), stop=(c == CH - 1),
            )
        rnkT = consts.tile([H, t_dim], F32, tag="rnkT")
        nc.scalar.sqrt(rnkT, psk2)
        nc.vector.reciprocal(rnkT, rnkT)

        # Kmat[d, t, h] = kT[d, t] * m48[d, h];  Vsel[d, t, h] = vT[d, t] * mv[d, h]
        kmat = consts.tile([P, CH, t_dim, H], CD, tag="kmat")
        nc.vector.tensor_tensor(
            kmat,
            kT[:, :, :, None].to_broadcast((P, CH, t_dim, H)),
            m48_sb[:, :, None, :].to_broadcast((P, CH, t_dim, H)),
            op=mybir.AluOpType.mult,
        )
        vsel = consts.tile([P, CH, t_dim, H], CD, tag="vsel")
        nc.vector.tensor_tensor(
            vsel,
            vT[:, :, :, None].to_broadcast((P, CH, t_dim, H)),
            mv_sb[:, :, None, :].to_broadcast((P, CH, t_dim, H)),
            op=mybir.AluOpType.mult,
        )

        # Wqk[d_in, t, h] = sum_dmid Wq[d_in, dmid] Kmat[dmid, t, h]
        wqk = consts.tile([P, CH, t_dim, H], CD, tag="wqk")
        for ci in range(CH):
            pw = psum2.tile([P, t_dim * H], F32, tag="par")
            for cm in range(CH):
                nc.tensor.matmul(
                    pw,
                    wqT_sb[:, cm, ci * P : (ci + 1) * P],
                    kmat[:, cm, :, :],
                    start=(cm == 0), stop=(cm == CH - 1),
                )
            nc.scalar.copy(wqk[:, ci], pw.rearrange("p (t h) -> p t h", h=H))

        # Wp_eff[(t,h), d_out] = sum_d Vsel[d, t, h] * Wp[d, d_out]  (all t at once)
        pe_all = psum3.tile([t_dim * H, D], F32, tag="po")
        for c in range(CH):
            nc.tensor.matmul(
                pe_all, vsel[:, c].rearrange("p t h -> p (t h)"), wp_sb[:, c, :],
                start=(c == 0), stop=(c == CH - 1),
            )
        wpe_stage = work.tile([t_dim * H, D], CD, tag="wpestage")
        nc.scalar.copy(wpe_stage, pe_all)
        wpe_dram = dram.tile([t_dim * H, D], CD)
        nc.sync.dma_start(wpe_dram, wpe_stage)
        wpe = consts.tile([H, t_dim, D], CD, tag="wpe")
        nc.sync.dma_start(wpe, wpe_dram.rearrange("(t h) d -> h t d", h=H))

        # ---------- main loop (software-pipelined: tail lags head by 1 group) ----------
        def emit_tail(par, qsq, t, sl):
            # per-head sum of squares -> rows 32:40 (inputs computed last group)
            for co in range(CH):
                nc.tensor.matmul(
                    par[32 : 32 + H, :], m48_sb[:, co, :], qsq[:, co, :],
                    start=(co == 0), stop=(co == CH - 1),
                    tile_position=(0, 32),
                )
            nrm = small.tile([H, ng], F32, tag="nrm")
            nc.scalar.sqrt(nrm, par[32 : 32 + H, :])
            rcp = small.tile([H, ng], F32, tag="rcp")
            nc.vector.reciprocal_approx_fast(rcp, nrm)
            att = small.tile([H, ng], CD, tag="att")
            nc.vector.scalar_tensor_tensor(
                att, par[0:H, :], rnkT[:, t : t + 1], rcp,
                op0=mybir.AluOpType.mult, op1=mybir.AluOpType.mult,
            )
            # out^T chunks = Wp_eff^T @ attn + bp
            osb = work.tile([P, CH, ng], F32, tag="osb")
            for co in range(CH):
                po = psum3.tile([P, ng], F32, tag="po")
                nc.tensor.matmul(
                    po, wpe[:, t, co * P : (co + 1) * P], att,
                    start=True, stop=True,
                )
                if co == 0:
                    nc.scalar.activation(
                        osb[:, co, :], po,
                        mybir.ActivationFunctionType.Identity,
                        bias=bp_sb[:, co : co + 1], scale=1.0,
                    )
                else:
                    nc.vector.tensor_tensor(
                        osb[:, co, :], po,
                        bp_sb[:, co : co + 1].to_broadcast((P, ng)),
                        op=mybir.AluOpType.add,
                    )
            nc.sync.dma_start(
                outT[t].rearrange("(c p) n -> p c n", p=P)[:, :, sl], osb
            )

        pending = None
        for t in range(t_dim):
            qt = qt0 if t == 0 else qpool.tile([P, CH, n_dim], CD, tag="qt")
            if t > 0:
                for c in range(CH):
                    nc.sync.dma_start(qt[:, c, :], qT[t, c * P : (c + 1) * P, :])
            for g in range(ngrp):
                sl = slice(g * ng, (g + 1) * ng)
                # projection chunks; per-chunk square + ss matmul interleave.
                # ss -> rows 32:40, attn_raw -> rows 0:8 of one psum bank.
                par = psum2.tile([40, ng], F32, tag="par")
                pq = psum1.tile([P, CH, ng], F32, tag="pq")
                qsq = work.tile([P, CH, ng], CD, tag="qsq")
                for co in range(CH):
                    for c in range(CH):
                        nc.tensor.matmul(
                            pq[:, co, :],
                            wq_sb[:, c, co * P : (co + 1) * P], qt[:, c, sl],
                            start=(c == 0), stop=(c == CH - 1),
                        )
                    nc.scalar.square(qsq[:, co, :], pq[:, co, :])
                for c in range(CH):
                    nc.tensor.matmul(
                        par[0:H, :], wqk[:, c, t, :], qt[:, c, sl],
                        start=(c == 0), stop=(c == CH - 1),
                        tile_position=(0, 0),
                    )
                if pending is not None:
                    emit_tail(*pending)
                pending = (par, qsq, t, sl)
        emit_tail(*pending)

    nc.compile()
    return nc


_CACHE = {}


def _get_nc(t_dim=T, n_dim=N):
    key = (t_dim, n_dim, COMPUTE_DTYPE)
    if key not in _CACHE:
        _CACHE[key] = build_nc(t_dim, n_dim)
    return _CACHE[key]


def _host_prep(query, kv, Wq, Wk, Wv, Wp, bp):
    ndt = _NPDT[COMPUTE_DTYPE]
    query = np.asarray(query, dtype=np.float32)
    kv = np.asarray(kv, dtype=np.float32)
    Wq = np.ascontiguousarray(np.asarray(Wq, dtype=np.float32).astype(ndt))
    WqT = np.ascontiguousarray(Wq.T)
    Wk = np.ascontiguousarray(np.asarray(Wk, dtype=np.float32).astype(ndt))
    Wv = np.asarray(Wv, dtype=np.float32)
    Wp = np.ascontiguousarray(np.asarray(Wp, dtype=np.float32).astype(ndt))
    bp = np.ascontiguousarray(np.asarray(bp, dtype=np.float32))

    b_dim, t_dim, n_dim, d = query.shape
    dh = d // H
    # Wv with columns permuted: v_perm[d] = v[(d%H)*dh + d//H]
    perm = (np.arange(d) % H) * dh + np.arange(d) // H
    Wvp = np.ascontiguousarray(Wv[:, perm].astype(ndt))
    dd = np.arange(d)
    hh = np.arange(H)
    m48 = (dd[:, None] // dh == hh[None, :]).astype(ndt)
    mv = (dd[:, None] % H == hh[None, :]).astype(ndt)

    in_maps = []
    for b in range(b_dim):
        in_maps.append(
            {
                "qT": np.ascontiguousarray(query[b].transpose(0, 2, 1).astype(ndt)),
                "kvT": np.ascontiguousarray(kv[b, :, 0, :].T.astype(ndt)),
                "Wq": Wq,
                "WqT": WqT,
                "Wk": Wk,
                "Wvp": Wvp,
                "Wp": Wp,
                "bp": bp,
                "m48": m48,
                "mv": mv,
            }
        )
    return in_maps, (b_dim, t_dim, n_dim, d)


def _gather(results, shape):
    b_dim, t_dim, n_dim, d = shape
    out = np.empty((b_dim, t_dim, n_dim, d), dtype=np.float32)
    for b in range(b_dim):
        out[b] = results[b]["outT"].transpose(0, 2, 1)
    return out


def kernel(query, kv, Wq, Wk, Wv, Wp, bp):
    from concourse.bass_utils import run_bass_kernel_spmd

    in_maps, shape = _host_prep(query, kv, Wq, Wk, Wv, Wp, bp)
    nc = _get_nc(shape[1], shape[2])
    res = run_bass_kernel_spmd(nc, in_maps, core_ids=list(range(len(in_maps))))
    return _gather(res.results, shape)


def _install_ntff_hook():
    """The agent image's antenv lacks axon_hooks; synthesize it so
    run_bass_kernel_spmd(trace=True) can capture NTFF profiles."""
    import types

    if "antenv.axon_hooks" in sys.modules:
        return
    sys.path.insert(0, "/root/.axon_site")
    from trn_agent_boot.trn_boot import _ntff_profile_via_ctypes

    hook = _ntff_profile_via_ctypes("/opt/axon/libaxon_pjrt.so")
    mod = types.ModuleType("antenv.axon_hooks")
    mod.get_axon_ntff_profile_hook = lambda: hook
    mod.set_axon_ntff_profile_hook = lambda h: None
    sys.modules["antenv.axon_hooks"] = mod


def kernel_traced(query, kv, Wq, Wk, Wv, Wp, bp):
    """Like kernel() but captures an NTFF profile; returns (out, results)."""
    from concourse.bass_utils import run_bass_kernel_spmd

    _install_ntff_hook()
    in_maps, shape = _host_prep(query, kv, Wq, Wk, Wv, Wp, bp)
    nc = _get_nc(shape[1], shape[2])
    res = run_bass_kernel_spmd(
        nc, in_maps, core_ids=list(range(len(in_maps))), trace=True
    )
    return _gather(res.results, shape), res


# revision 18
# speedup vs baseline: 1.3116x; 1.1158x over previous
"""CosineCrossAttention Trainium2 kernel.

Math (per (b,t)):
    q = query @ Wq                      (N, D), heads head-major: d = h*48+dh
    k = kv @ Wk   (1, D);  v = kv @ Wv  (1, D)
    attn[n,h] = (q_h . k_h) / (|q_h||k_h|)
    out[n, dh*8+h] = attn[n,h] * v[h,dh];  out = out @ Wp + bp

Restructured:
    k_scaled = k / |k_h|  per head
    Kmat[d,h]  = k_scaled[d] * (d//48 == h)          (D, H)
    Wqk        = Wq @ Kmat                           (D, H)
    attn_raw   = query @ Wqk        = q.k/|k|        (N, H)
    ss[n,h]    = sum_{d in head h} q[n,d]^2          via mask48 matmul on q^2
    attn       = attn_raw / sqrt(ss)
    v_perm     = kv @ Wv_perm   (Wv columns permuted so v_perm[d] = v[(d%8)*48+d//8])
    Wp_eff[h,:]= sum_d v_perm[d]*(d%8==h)*Wp[d,:]    (H, D)
    out        = attn @ Wp_eff + bp

Everything on-device runs in the transposed domain (D on partitions, N on free):
host passes query^T per (b,t) so matmul contraction dims land on partitions with
contiguous DMA. Sharding: data-parallel over B across the 8 cores.

COMPUTE_DTYPE selects the TensorEngine dtype: "bf16" (fast, ~1e-2 err),
"f32r" (full-rate fp32 streaming mode), or "f32" (exact, 4x slower PE).
PSUM accumulation is always fp32; norms/recip and the output are fp32.
"""

import sys

sys.path.insert(0, "/opt/trn_rl_repo")

from contextlib import ExitStack

import ml_dtypes
import numpy as np

import concourse.bass as bass
import concourse.tile as tile
from concourse import bacc, mybir
from concourse.masks import make_identity

F32 = mybir.dt.float32

B, T, N, D, H, Dh = 8, 8, 2048, 384, 8, 48
P = 128
CH = D // P  # 3 chunks of the D dims
NG = 512  # n-group (one PSUM bank of f32)

COMPUTE_DTYPE = "bf16"  # "bf16" | "f32r" | "f32"

_CDT = {
    "bf16": mybir.dt.bfloat16,
    "f32r": mybir.dt.float32r,
    "f32": mybir.dt.float32,
}
_NPDT = {
    "bf16": ml_dtypes.bfloat16,
    "f32r": np.float32,
    "f32": np.float32,
}


def build_nc(t_dim=T, n_dim=N, ng=NG, cdtype=None):
    cdtype = cdtype or COMPUTE_DTYPE
    CD = _CDT[cdtype]
    ngrp = n_dim // ng
    nc = bacc.Bacc("TRN2", target_bir_lowering=False, debug=False)

    qT = nc.dram_tensor("qT", [t_dim, D, n_dim], CD, kind="ExternalInput").ap()
    kvT = nc.dram_tensor("kvT", [D, t_dim], CD, kind="ExternalInput").ap()
    wq_d = nc.dram_tensor("Wq", [D, D], CD, kind="ExternalInput").ap()
    wqT_d = nc.dram_tensor("WqT", [D, D], CD, kind="ExternalInput").ap()
    wk_d = nc.dram_tensor("Wk", [D, D], CD, kind="ExternalInput").ap()
    wv_d = nc.dram_tensor("Wvp", [D, D], CD, kind="ExternalInput").ap()
    wp_d = nc.dram_tensor("Wp", [D, D], CD, kind="ExternalInput").ap()
    bp_d = nc.dram_tensor("bp", [D], F32, kind="ExternalInput").ap()
    m48_d = nc.dram_tensor("m48", [D, H], CD, kind="ExternalInput").ap()
    mv_d = nc.dram_tensor("mv", [D, H], CD, kind="ExternalInput").ap()
    outT = nc.dram_tensor("outT", [t_dim, D, n_dim], F32, kind="ExternalOutput").ap()

    with tile.TileContext(nc) as tc, ExitStack() as ctx:
        consts = ctx.enter_context(tc.tile_pool(name="consts", bufs=1))
        qpool = ctx.enter_context(tc.tile_pool(name="qpool", bufs=2))
        work = ctx.enter_context(tc.tile_pool(name="work", bufs=2))
        small = ctx.enter_context(tc.tile_pool(name="small", bufs=3))
        psum1 = ctx.enter_context(tc.tile_pool(name="psum1", bufs=4, space="PSUM"))
        psum2 = ctx.enter_context(tc.tile_pool(name="psum2", bufs=2, space="PSUM"))
        psum3 = ctx.enter_context(tc.tile_pool(name="psum3", bufs=2, space="PSUM"))
        dram = ctx.enter_context(tc.tile_pool(name="dram", bufs=1, space="DRAM"))

        # ---------- constants (small, first in the DMA queue) ----------
        def load_w(dram, tag):
            sb = consts.tile([P, CH, D], CD, tag=tag)
            nc.sync.dma_start(sb, dram.rearrange("(c p) f -> p c f", p=P))
            return sb

        wq_sb = load_w(wq_d, "wq")
        wk_sb = load_w(wk_d, "wk")
        wv_sb = load_w(wv_d, "wv")
        wp_sb = load_w(wp_d, "wp")

        m48_sb = consts.tile([P, CH, H], CD, tag="m48")
        nc.sync.dma_start(m48_sb, m48_d.rearrange("(c p) h -> p c h", p=P))
        mv_sb = consts.tile([P, CH, H], CD, tag="mv")
        nc.sync.dma_start(mv_sb, mv_d.rearrange("(c p) h -> p c h", p=P))
        kvt_sb = consts.tile([P, CH, t_dim], CD, tag="kvt")
        nc.sync.dma_start(kvt_sb, kvT.rearrange("(c p) t -> p c t", p=P))
        bp_sb = consts.tile([P, CH], F32, tag="bp")
        nc.sync.dma_start(bp_sb, bp_d.rearrange("(c p) -> p c", p=P))

        # Wq^T comes pre-transposed from the host
        wqT_sb = load_w(wqT_d, "wqT")

        # ---------- k/v projections for all t ----------
        ps_k = psum2.tile([t_dim, D], F32, tag="par")
        ps_v = psum2.tile([t_dim, D], F32, tag="par")
        for c in range(CH):
            nc.tensor.matmul(
                ps_k, kvt_sb[:, c, :], wk_sb[:, c, :],
                start=(c == 0), stop=(c == CH - 1),
            )
        for c in range(CH):
            nc.tensor.matmul(
                ps_v, kvt_sb[:, c, :], wv_sb[:, c, :],
                start=(c == 0), stop=(c == CH - 1),
            )

        # copies to SBUF (k left unscaled; 1/|k_h| is folded into the tail)
        k_sb = work.tile([t_dim, D], F32, tag="k_sb")
        nc.scalar.copy(k_sb, ps_k)
        vsb = work.tile([t_dim, D], F32, tag="vsb")
        nc.scalar.copy(vsb, ps_v)

        # transpose k, v -> (D-part, t); cast to compute dtype on copyback
        kT = consts.tile([P, CH, t_dim], CD, tag="kT")
        vT = consts.tile([P, CH, t_dim], CD, tag="vT")
        idt = consts.tile([t_dim, t_dim], F32, tag="idt")
        make_identity(nc, idt)
        for c in range(CH):
            pt = psum1.tile([P, t_dim], F32, tag="pq")
            nc.tensor.transpose(pt, k_sb[:, c * P : (c + 1) * P], idt)
            nc.vector.tensor_copy(kT[:, c, :], pt)
            pt2 = psum3.tile([P, t_dim], F32, tag="po")
            nc.tensor.transpose(pt2, vsb[:, c * P : (c + 1) * P], idt)
            nc.vector.tensor_copy(vT[:, c, :], pt2)

        # per-head k norms, transposed domain (off the wqk critical path):
        # rnkT[h, t] = 1/|k_h|(t)
        ksqT = work.tile([P, CH, t_dim], CD, tag="ksqT")
        nc.scalar.square(ksqT, kT)
        psk2 = psum3.tile([H, t_dim], F32, tag="po")
        for c in range(CH):
            nc.tensor.matmul(
                psk2, m48_sb[:, c, :], ksqT[:, c, :],
                start=(c == 0), stop=(c == CH - 1),
            )
        rnkT = consts.tile([H, t_dim], F32, tag="rnkT")
        nc.scalar.sqrt(rnkT, psk2)
        nc.vector.reciprocal(rnkT, rnkT)

        # Kmat[d, t, h] = kT[d, t] * m48[d, h];  Vsel[d, t, h] = vT[d, t] * mv[d, h]
        kmat = consts.tile([P, CH, t_dim, H], CD, tag="kmat")
        nc.vector.tensor_tensor(
            kmat,
            kT[:, :, :, None].to_broadcast((P, CH, t_dim, H)),
            m48_sb[:, :, None, :].to_broadcast((P, CH, t_dim, H)),
            op=mybir.AluOpType.mult,
        )
        vsel = consts.tile([P, CH, t_dim, H], CD, tag="vsel")
        nc.vector.tensor_tensor(
            vsel,
            vT[:, :, :, None].to_broadcast((P, CH, t_dim, H)),
            mv_sb[:, :, None, :].to_broadcast((P, CH, t_dim, H)),
            op=mybir.AluOpType.mult,
        )

        # Wqk[d_in, t, h] = sum_dmid Wq[d_in, dmid] Kmat[dmid, t, h]
        wqk = consts.tile([P, CH, t_dim, H], CD, tag="wqk")
        for ci in range(CH):
            pw = psum2.tile([P, t_dim * H], F32, tag="par")
            for cm in range(CH):
                nc.tensor.matmul(
                    pw,
                    wqT_sb[:, cm, ci * P : (ci + 1) * P],
                    kmat[:, cm, :, :],
                    start=(cm == 0), stop=(cm == CH - 1),
                )
            nc.scalar.copy(wqk[:, ci], pw.rearrange("p (t h) -> p t h", h=H))

        # Wp_eff[(t,h), d_out] = sum_d Vsel[d, t, h] * Wp[d, d_out]  (all t at once)
        pe_all = psum3.tile([t_dim * H, D], F32, tag="po")
        for c in range(CH):
            nc.tensor.matmul(
                pe_all, vsel[:, c].rearrange("p t h -> p (t h)"), wp_sb[:, c, :],
                start=(c == 0), stop=(c == CH - 1),
            )
        wpe_stage = work.tile([t_dim * H, D], CD, tag="wpestage")
        nc.scalar.copy(wpe_stage, pe_all)
        wpe_dram = dram.tile([t_dim * H, D], CD)
        nc.sync.dma_start(wpe_dram, wpe_stage)
        wpe = consts.tile([H, t_dim, D], CD, tag="wpe")
        nc.sync.dma_start(wpe, wpe_dram.rearrange("(t h) d -> h t d", h=H))

        # ---------- main loop (software-pipelined: tail lags head by 1 group) ----------
        def emit_tail(par, qsq, t, sl):
            # per-head sum of squares -> rows 32:40 (inputs computed last group)
            for co in range(CH):
                nc.tensor.matmul(
                    par[32 : 32 + H, :], m48_sb[:, co, :], qsq[:, co, :],
                    start=(co == 0), stop=(co == CH - 1),
                    tile_position=(0, 32),
                )
            nrm = small.tile([H, ng], F32, tag="nrm")
            nc.scalar.sqrt(nrm, par[32 : 32 + H, :])
            rcp = small.tile([H, ng], F32, tag="rcp")
            nc.vector.reciprocal_approx_fast(rcp, nrm)
            att = small.tile([H, ng], CD, tag="att")
            nc.vector.scalar_tensor_tensor(
                att, par[0:H, :], rnkT[:, t : t + 1], rcp,
                op0=mybir.AluOpType.mult, op1=mybir.AluOpType.mult,
            )
            # out^T chunks = Wp_eff^T @ attn + bp
            osb = work.tile([P, CH, ng], F32, tag="osb")
            for co in range(CH):
                po = psum3.tile([P, ng], F32, tag="po")
                nc.tensor.matmul(
                    po, wpe[:, t, co * P : (co + 1) * P], att,
                    start=True, stop=True,
                )
                if co == 0:
                    nc.scalar.activation(
                        osb[:, co, :], po,
                        mybir.ActivationFunctionType.Identity,
                        bias=bp_sb[:, co : co + 1], scale=1.0,
                    )
                else:
                    nc.vector.tensor_tensor(
                        osb[:, co, :], po,
                        bp_sb[:, co : co + 1].to_broadcast((P, ng)),
                        op=mybir.AluOpType.add,
                    )
            nc.sync.dma_start(
                outT[t].rearrange("(c p) n -> p c n", p=P)[:, :, sl], osb
            )

        pending = None
        qsplit = 2 if ngrp >= 2 else 1
        gph = ngrp // qsplit  # groups per query-slice tile
        nh = gph * ng
        for t in range(t_dim):
            for hf in range(qsplit):
                qt = qpool.tile([P, CH, nh], CD, tag="qt")
                for c in range(CH):
                    nc.sync.dma_start(
                        qt[:, c, :],
                        qT[t, c * P : (c + 1) * P, hf * nh : (hf + 1) * nh],
                    )
                for gl in range(gph):
                    g = hf * gph + gl
                    sl = slice(g * ng, (g + 1) * ng)
                    qsl = slice(gl * ng, (gl + 1) * ng)
                    # attn_raw -> rows 0:8, ss -> rows 32:40 of one psum bank
                    par = psum2.tile([40, ng], F32, tag="par")
                    qsq = work.tile([P, CH, ng], CD, tag="qsq")
                    for co in range(CH):
                        pqc = psum1.tile([P, ng], F32, tag="pq")
                        for c in range(CH):
                            nc.tensor.matmul(
                                pqc,
                                wq_sb[:, c, co * P : (co + 1) * P], qt[:, c, qsl],
                                start=(c == 0), stop=(c == CH - 1),
                            )
                        nc.scalar.square(qsq[:, co, :], pqc)
                    for c in range(CH):
                        nc.tensor.matmul(
                            par[0:H, :], wqk[:, c, t, :], qt[:, c, qsl],
                            start=(c == 0), stop=(c == CH - 1),
                            tile_position=(0, 0),
                        )
                    if pending is not None:
                        emit_tail(*pending)
                    pending = (par, qsq, t, sl)
        emit_tail(*pending)

    nc.compile()
    return nc


_CACHE = {}


def _get_nc(t_dim=T, n_dim=N):
    key = (t_dim, n_dim, COMPUTE_DTYPE)
    if key not in _CACHE:
        _CACHE[key] = build_nc(t_dim, n_dim)
    return _CACHE[key]


def _host_prep(query, kv, Wq, Wk, Wv, Wp, bp):
    ndt = _NPDT[COMPUTE_DTYPE]
    query = np.asarray(query, dtype=np.float32)
    kv = np.asarray(kv, dtype=np.float32)
    Wq = np.ascontiguousarray(np.asarray(Wq, dtype=np.float32).astype(ndt))
    WqT = np.ascontiguousarray(Wq.T)
    Wk = np.ascontiguousarray(np.asarray(Wk, dtype=np.float32).astype(ndt))
    Wv = np.asarray(Wv, dtype=np.float32)
    Wp = np.ascontiguousarray(np.asarray(Wp, dtype=np.float32).astype(ndt))
    bp = np.ascontiguousarray(np.asarray(bp, dtype=np.float32))

    b_dim, t_dim, n_dim, d = query.shape
    dh = d // H
    # Wv with columns permuted: v_perm[d] = v[(d%H)*dh + d//H]
    perm = (np.arange(d) % H) * dh + np.arange(d) // H
    Wvp = np.ascontiguousarray(Wv[:, perm].astype(ndt))
    dd = np.arange(d)
    hh = np.arange(H)
    m48 = (dd[:, None] // dh == hh[None, :]).astype(ndt)
    mv = (dd[:, None] % H == hh[None, :]).astype(ndt)

    in_maps = []
    for b in range(b_dim):
        in_maps.append(
            {
                "qT": np.ascontiguousarray(query[b].transpose(0, 2, 1).astype(ndt)),
                "kvT": np.ascontiguousarray(kv[b, :, 0, :].T.astype(ndt)),
                "Wq": Wq,
                "WqT": WqT,
                "Wk": Wk,
                "Wvp": Wvp,
                "Wp": Wp,
                "bp": bp,
                "m48": m48,
                "mv": mv,
            }
        )
    return in_maps, (b_dim, t_dim, n_dim, d)


def _gather(results, shape):
    b_dim, t_dim, n_dim, d = shape
    out = np.empty((b_dim, t_dim, n_dim, d), dtype=np.float32)
    for b in range(b_dim):
        out[b] = results[b]["outT"].transpose(0, 2, 1)
    return out


def kernel(query, kv, Wq, Wk, Wv, Wp, bp):
    from concourse.bass_utils import run_bass_kernel_spmd

    in_maps, shape = _host_prep(query, kv, Wq, Wk, Wv, Wp, bp)
    nc = _get_nc(shape[1], shape[2])
    res = run_bass_kernel_spmd(nc, in_maps, core_ids=list(range(len(in_maps))))
    return _gather(res.results, shape)


def _install_ntff_hook():
    """The agent image's antenv lacks axon_hooks; synthesize it so
    run_bass_kernel_spmd(trace=True) can capture NTFF profiles."""
    import types

    if "antenv.axon_hooks" in sys.modules:
        return
    sys.path.insert(0, "/root/.axon_site")
    from trn_agent_boot.trn_boot import _ntff_profile_via_ctypes

    hook = _ntff_profile_via_ctypes("/opt/axon/libaxon_pjrt.so")
    mod = types.ModuleType("antenv.axon_hooks")
    mod.get_axon_ntff_profile_hook = lambda: hook
    mod.set_axon_ntff_profile_hook = lambda h: None
    sys.modules["antenv.axon_hooks"] = mod


def kernel_traced(query, kv, Wq, Wk, Wv, Wp, bp):
    """Like kernel() but captures an NTFF profile; returns (out, results)."""
    from concourse.bass_utils import run_bass_kernel_spmd

    _install_ntff_hook()
    in_maps, shape = _host_prep(query, kv, Wq, Wk, Wv, Wp, bp)
    nc = _get_nc(shape[1], shape[2])
    res = run_bass_kernel_spmd(
        nc, in_maps, core_ids=list(range(len(in_maps))), trace=True
    )
    return _gather(res.results, shape), res
